# revision 1
# baseline (speedup 1.0000x reference)
"""MoE FFN (grouped top-1 routing, SwiGLU experts) on 8 Trainium2 NeuronCores.

Strategy (expert-parallel, per sharding hint):
  - Host computes the (tiny) routers: sigmoid(x @ macro_w) -> top-1 group of 4;
    within the selected group both 2 experts are active (TOP_K==EXPERTS_PER_GROUP)
    with sigmoid-normalized weights. Router cost is ~25 MFLOP -> negligible.
  - Tokens are dispatched by routed group ("all-to-all" staged host-side into
    per-core input maps). Core c owns expert c (group c//2); it receives the
    tokens of its group, padded to capacity C, plus its expert's weights.
  - Per-expert weight w[t,e] is folded into the up-projection input on the host
    (x*w), so the device output is already weighted; host just adds the two
    expert partials of each group and scatters back to token order.
  - Device kernel: Y^T = down^T @ (silu(gate^T X^T) * (up^T Xw^T)), all with
    features on SBUF partitions and tokens on the free dim, fp32 storage with
    float32r (FP22) matmuls on the PE.
"""

import math

import ml_dtypes
import numpy as np

import concourse.bass as bass  # noqa: F401  (bass types via bacc)
import concourse.mybir as mybir
import concourse.tile as tile
from concourse import bacc
from concourse.bass_utils import run_bass_kernel_spmd

P = 128
D_MODEL = 1024
FFN_DIM = 2048
NUM_EXPERTS = 8
NUM_GROUPS = 4
EPS = 1e-9

F32 = mybir.dt.float32
F32R = mybir.dt.float32r
BF16 = mybir.dt.bfloat16

N_CORES = 8
C_CAP = 1024  # max token capacity per core per round (SBUF-bounded)

_BUILD_CACHE: dict[int, object] = {}
LAST_RESULTS = None  # stashed BassKernelResults for test harnesses


def _build(C: int, nch: int):
    """Bass/Tile program for one expert: [D,C]x2 tokens + expert weights -> [D,C]."""
    chunk = C // nch
    assert chunk * nch == C and chunk <= 512
    DO = D_MODEL // P  # 8 k-tiles over D
    FO = FFN_DIM // P  # 16 f-tiles over F

    nc = bacc.Bacc(
        "TRN2",
        target_bir_lowering=False,
        debug=False,
        enable_asserts=False,
        num_devices=N_CORES,
    )
    xt = nc.dram_tensor("xt", [D_MODEL, C], BF16, kind="ExternalInput").ap()
    xwt = nc.dram_tensor("xwt", [D_MODEL, C], BF16, kind="ExternalInput").ap()
    gw = nc.dram_tensor("gw", [D_MODEL, FFN_DIM], BF16, kind="ExternalInput").ap()
    uw = nc.dram_tensor("uw", [D_MODEL, FFN_DIM], BF16, kind="ExternalInput").ap()
    dw = nc.dram_tensor("dw", [FFN_DIM, D_MODEL], BF16, kind="ExternalInput").ap()
    yt = nc.dram_tensor("yt", [D_MODEL, C], F32, kind="ExternalOutput").ap()

    gwr = gw.rearrange("(do p) f -> p do f", p=P)
    uwr = uw.rearrange("(do p) f -> p do f", p=P)
    dwr = dw.rearrange("(fo p) d -> p fo d", p=P)
    xtr = xt.rearrange("(do p) c -> p do c", p=P)
    xwtr = xwt.rearrange("(do p) c -> p do c", p=P)

    csl = [slice(cc * chunk, (cc + 1) * chunk) for cc in range(nch)]
    with tile.TileContext(nc) as tc:
        with (
            tc.tile_pool(name="xp", bufs=1) as xp,
            tc.tile_pool(name="wp", bufs=3) as wp,
            tc.tile_pool(name="dp", bufs=3) as dp,
            tc.tile_pool(name="hp", bufs=1) as hp,
            tc.tile_pool(name="sp", bufs=4) as sp,
            tc.tile_pool(name="yp", bufs=4) as yp,
            tc.tile_pool(name="pg", bufs=2, space="PSUM") as pgp,
            tc.tile_pool(name="pu", bufs=2, space="PSUM") as pup,
            tc.tile_pool(name="pd", bufs=3, space="PSUM") as pdp,
        ):
            # Wide weight DMAs (4 f-tiles per transfer) on the sync queue;
            # token loads on gpsimd so they don't delay the weight stream.
            NW = 4
            gt4s = {}
            ut4s = {}
            for fw in range(2):
                gt4s[fw] = wp.tile([P, DO, NW * P], BF16, tag="gt", name=f"gt4_{fw}")
                nc.sync.dma_start(gt4s[fw][:], gwr[:, :, fw * NW * P : (fw + 1) * NW * P])
                ut4s[fw] = wp.tile([P, DO, NW * P], BF16, tag="ut", name=f"ut4_{fw}")
                nc.sync.dma_start(ut4s[fw][:], uwr[:, :, fw * NW * P : (fw + 1) * NW * P])
            xts = xp.tile([P, DO, C], BF16, tag="xt")
            xws = xp.tile([P, DO, C], BF16, tag="xw")
            for do in range(DO):
                nc.gpsimd.dma_start(xts[:, do], xtr[:, do])
                nc.gpsimd.dma_start(xws[:, do], xwtr[:, do])
            hs = hp.tile([P, FO, C], BF16, tag="h")

            for fw in range(FO // NW):
                if fw not in gt4s:
                    gt4s[fw] = wp.tile([P, DO, NW * P], BF16, tag="gt", name=f"gt4_{fw}")
                    nc.sync.dma_start(
                        gt4s[fw][:], gwr[:, :, fw * NW * P : (fw + 1) * NW * P]
                    )
                    ut4s[fw] = wp.tile([P, DO, NW * P], BF16, tag="ut", name=f"ut4_{fw}")
                    nc.sync.dma_start(
                        ut4s[fw][:], uwr[:, :, fw * NW * P : (fw + 1) * NW * P]
                    )
                gt4, ut4 = gt4s[fw], ut4s[fw]
                for fl in range(NW):
                    fo = fw * NW + fl
                    fsl = slice(fl * P, (fl + 1) * P)
                    for cc in range(nch):
                        cs = csl[cc]
                        psg = pgp.tile([P, chunk], F32, tag="psg", name=f"psg_{fo}_{cc}")
                        psu = pup.tile([P, chunk], F32, tag="psu", name=f"psu_{fo}_{cc}")
                        for do in range(DO):
                            nc.tensor.matmul(
                                psg[:],
                                gt4[:, do, fsl],
                                xts[:, do, cs],
                                start=(do == 0),
                                stop=(do == DO - 1),
                            )
                        for do in range(DO):
                            nc.tensor.matmul(
                                psu[:],
                                ut4[:, do, fsl],
                                xws[:, do, cs],
                                start=(do == 0),
                                stop=(do == DO - 1),
                            )
                        sg = sp.tile([P, chunk], F32, tag="sg")
                        nc.scalar.activation(
                            sg[:], psg[:], mybir.ActivationFunctionType.Silu
                        )
                        nc.vector.tensor_mul(out=hs[:, fo, cs], in0=sg[:], in1=psu[:])

            for do in range(DO):
                dt_ = dp.tile([P, FO, P], BF16, tag="dt")
                nc.sync.dma_start(dt_[:], dwr[:, :, do * P : (do + 1) * P])
                for cc in range(nch):
                    cs = csl[cc]
                    psy = pdp.tile([P, chunk], F32, tag="psy", name=f"psy_{do}_{cc}")
                    for fo in range(FO):
                        nc.tensor.matmul(
                            psy[:],
                            dt_[:, fo],
                            hs[:, fo, cs],
                            start=(fo == 0),
                            stop=(fo == FO - 1),
                        )
                    yo = yp.tile([P, chunk], F32, tag="yo")
                    nc.any.tensor_copy(out=yo[:], in_=psy[:])
                    nc.gpsimd.dma_start(yt[do * P : (do + 1) * P, cs], yo[:])
    nc.finalize()
    return nc


def _get_program(C: int, nch: int):
    key = (C, nch)
    if key not in _BUILD_CACHE:
        _BUILD_CACHE[key] = _build(C, nch)
    return _BUILD_CACHE[key]


def _sigmoid(z):
    return 1.0 / (1.0 + np.exp(-z))


def _route(xf32, macro_w, micro_w):
    """Host routers in float64. Returns group index per token and per-token
    weights for the 2 experts of the selected group (float32)."""
    xf = xf32.astype(np.float64)
    ms = _sigmoid(xf @ macro_w.astype(np.float64))  # [T, G]
    g_sel = np.argmax(ms, axis=1)
    T = xf.shape[0]
    mval = ms[np.arange(T), g_sel]
    mv = mval / (mval + EPS)

    w2 = np.zeros((T, 2), np.float64)
    for g in range(NUM_GROUPS):
        idx = np.nonzero(g_sel == g)[0]
        if idx.size == 0:
            continue
        s = _sigmoid(xf[idx] @ micro_w[g].astype(np.float64))  # [n, 2]
        denom = np.maximum(s[:, 0], s[:, 1]) + np.minimum(s[:, 0], s[:, 1]) + EPS
        w2[idx, 0] = mv[idx] * s[:, 0] / denom
        w2[idx, 1] = mv[idx] * s[:, 1] / denom
    return g_sel, w2.astype(np.float32)


def _pick_capacity(n: int):
    n = max(n, 64)
    nch = (n + 511) // 512
    chunk = -(-n // nch)
    chunk = -(-chunk // 16) * 16
    return chunk * nch, nch


def kernel(x, macro_w, micro_w, gate_w, up_w, down_w):
    global LAST_RESULTS
    x = np.asarray(x)
    B, S, D = x.shape
    T = B * S
    xf = np.ascontiguousarray(x.reshape(T, D).astype(np.float32, copy=False))

    g_sel, w2 = _route(xf, np.asarray(macro_w), np.asarray(micro_w))
    idx_by_g = [np.nonzero(g_sel == g)[0] for g in range(NUM_GROUPS)]
    max_n = max(ix.size for ix in idx_by_g)

    n_rounds = max(1, math.ceil(max_n / C_CAP))
    if n_rounds > 1:
        C, nch = C_CAP, 2
    else:
        C, nch = _pick_capacity(max_n)
    nc = _get_program(C, nch)

    gate_w = np.ascontiguousarray(np.asarray(gate_w, np.float32)).astype(ml_dtypes.bfloat16)
    up_w = np.ascontiguousarray(np.asarray(up_w, np.float32)).astype(ml_dtypes.bfloat16)
    down_w = np.ascontiguousarray(np.asarray(down_w, np.float32)).astype(ml_dtypes.bfloat16)

    y = np.zeros((T, D), np.float32)
    for r in range(n_rounds):
        in_maps = []
        round_idx = []
        for c in range(N_CORES):
            g = c // 2
            j = c % 2  # local expert within group
            ix = idx_by_g[g][r * C_CAP : r * C_CAP + C]
            round_idx.append(ix)
            xt = np.zeros((D, C), ml_dtypes.bfloat16)
            xwt = np.zeros((D, C), ml_dtypes.bfloat16)
            if ix.size:
                xg = xf[ix]
                xt[:, : ix.size] = xg.T.astype(ml_dtypes.bfloat16)
                xwt[:, : ix.size] = (xg * w2[ix, j : j + 1]).T.astype(ml_dtypes.bfloat16)
            in_maps.append(
                {
                    "xt": xt,
                    "xwt": xwt,
                    "gw": gate_w[c],
                    "uw": up_w[c],
                    "dw": down_w[c],
                }
            )
        res = run_bass_kernel_spmd(nc, in_maps, core_ids=list(range(N_CORES)))
        LAST_RESULTS = res
        for g in range(NUM_GROUPS):
            ix = round_idx[2 * g]
            if ix.size:
                ysum = res.results[2 * g]["yt"] + res.results[2 * g + 1]["yt"]
                y[ix] = ysum[:, : ix.size].T
    return y.reshape(B, S, D)



# revision 5
# speedup vs baseline: 1.0996x; 1.0996x over previous
"""MoE FFN (grouped top-1 routing, SwiGLU experts) on 8 Trainium2 NeuronCores.

Strategy (expert-parallel with half-expert load balancing):
  - Host computes the routers (sigmoid macro top-1 group of 4; both experts of
    the selected group active with normalized sigmoid weights). Per-expert
    weight folded into the up-projection input (x*w) on the host.
  - Each expert's FFN dim F=2048 is split into two half-experts (F_half=1024).
    The 16 half-experts are placed so every core gets one half-expert from a
    HEAVY group (task A) and one from a LIGHT group (task B): heavy groups are
    the two with most routed tokens. This balances per-core work to
    ~(s0+s2)/2 tokens instead of max-group tokens.
  - Device kernel per core: two independent SwiGLU half-FFN tasks,
    Y_h^T = down_h^T @ (silu(gate_h^T X^T) * (up_h^T Xw^T)). bf16 storage,
    fp32 PSUM accumulation. Host sums the 4 half-partials per token.
  - DMA is explicitly staged (tokens + first weight slabs first, down-proj
    weights prefetched during phase 1) and the PE is pre-warmed with dummy
    matmuls so the HAM clock gate reaches 2.4 GHz before real work arrives.
"""

import math

import ml_dtypes
import numpy as np

import concourse.bass as bass  # noqa: F401  (bass types via bacc)
import concourse.mybir as mybir
import concourse.tile as tile
from concourse import bacc
from concourse.bass_utils import run_bass_kernel_spmd

P = 128
D_MODEL = 1024
FFN_DIM = 2048
F_HALF = FFN_DIM // 2
NUM_EXPERTS = 8
NUM_GROUPS = 4
EPS = 1e-9
DO = D_MODEL // P  # 8 k-tiles over D
FO_H = F_HALF // P  # 8 f-tiles per half-expert

F32 = mybir.dt.float32
BF16 = mybir.dt.bfloat16

N_CORES = 8
N_WARM = 16  # PE warmup matmuls (HAM un-throttle)

_BUILD_CACHE: dict[tuple, object] = {}
LAST_RESULTS = None  # stashed BassKernelResults for test harnesses


def _task_phase1(nc, gt3, ut3, xts3, xws3, hs, pools, chunk, nch, tag):
    """SwiGLU up to h = silu(gate^T x) * (up^T xw) for one half-expert task."""
    sp, pgp, pup = pools
    for cc in range(nch):
        cs = slice(cc * chunk, (cc + 1) * chunk)
        for fo in range(FO_H):
            psg = pgp.tile([P, 512], F32, tag="psg", name=f"psg{tag}_{cc}_{fo}")[:, :chunk]
            psu = pup.tile([P, 512], F32, tag="psu", name=f"psu{tag}_{cc}_{fo}")[:, :chunk]
            for do in range(DO):
                nc.tensor.matmul(
                    psg[:],
                    gt3[:, fo * DO + do, :],
                    xts3[:, cc * DO + do, :],
                    start=(do == 0),
                    stop=(do == DO - 1),
                )
            for do in range(DO):
                nc.tensor.matmul(
                    psu[:],
                    ut3[:, fo * DO + do, :],
                    xws3[:, cc * DO + do, :],
                    start=(do == 0),
                    stop=(do == DO - 1),
                )
            sg = sp.tile([P, chunk], F32, tag=f"sg{tag}", name=f"sg{tag}_{cc}_{fo}")
            nc.scalar.activation(sg[:], psg[:], mybir.ActivationFunctionType.Silu)
            nc.vector.tensor_mul(out=hs[:, fo, cs], in0=sg[:], in1=psu[:])


def _task_phase2(nc, dt3, hs, yt3, pools, chunk, nch, tag):
    """y^T = down_h^T @ h for one half-expert task; stream bf16 output out."""
    yp, pdp = pools
    for cc in range(nch):
        cs = slice(cc * chunk, (cc + 1) * chunk)
        for do in range(DO):
            psy = pdp.tile([P, 512], F32, tag="psy", name=f"psy{tag}_{cc}_{do}")[:, :chunk]
            for fo in range(FO_H):
                nc.tensor.matmul(
                    psy[:],
                    dt3[:, do * FO_H + fo, :],
                    hs[:, fo, cs],
                    start=(fo == 0),
                    stop=(fo == FO_H - 1),
                )
            yo = yp.tile([P, chunk], BF16, tag=f"yo{tag}", name=f"yo{tag}_{cc}_{do}")
            nc.any.tensor_copy(out=yo[:], in_=psy[:])
            nc.scalar.dma_start(yt3[:, cc * DO + do, :], yo[:])


def _build(CA: int, nchA: int, CB: int, nchB: int):
    """Bass/Tile program: two half-expert SwiGLU tasks per core."""
    chunkA, chunkB = CA // nchA, CB // nchB
    assert chunkA * nchA == CA and chunkA <= 512
    assert chunkB * nchB == CB and chunkB <= 512

    nc = bacc.Bacc(
        "TRN2",
        target_bir_lowering=False,
        debug=False,
        enable_asserts=False,
        num_devices=N_CORES,
    )
    wk = nc.dram_tensor("wk", [P, P], BF16, kind="ExternalInput").ap()
    xtA = nc.dram_tensor("xtA", [P, DO * CA], BF16, kind="ExternalInput").ap()
    xwA = nc.dram_tensor("xwA", [P, DO * CA], BF16, kind="ExternalInput").ap()
    xtB = nc.dram_tensor("xtB", [P, DO * CB], BF16, kind="ExternalInput").ap()
    xwB = nc.dram_tensor("xwB", [P, DO * CB], BF16, kind="ExternalInput").ap()
    gwA = nc.dram_tensor("gwA", [P, FO_H * DO * P], BF16, kind="ExternalInput").ap()
    uwA = nc.dram_tensor("uwA", [P, FO_H * DO * P], BF16, kind="ExternalInput").ap()
    gwB = nc.dram_tensor("gwB", [P, FO_H * DO * P], BF16, kind="ExternalInput").ap()
    uwB = nc.dram_tensor("uwB", [P, FO_H * DO * P], BF16, kind="ExternalInput").ap()
    dwA = nc.dram_tensor("dwA", [P, DO * FO_H * P], BF16, kind="ExternalInput").ap()
    dwB = nc.dram_tensor("dwB", [P, DO * FO_H * P], BF16, kind="ExternalInput").ap()
    ytA = nc.dram_tensor("ytA", [P, DO * CA], BF16, kind="ExternalOutput").ap()
    ytB = nc.dram_tensor("ytB", [P, DO * CB], BF16, kind="ExternalOutput").ap()

    # packed views: weights [p, idx, 128], tokens/outputs [p, idx, chunk]
    gA3 = gwA.rearrange("p (x f) -> p x f", f=P)
    uA3 = uwA.rearrange("p (x f) -> p x f", f=P)
    gB3 = gwB.rearrange("p (x f) -> p x f", f=P)
    uB3 = uwB.rearrange("p (x f) -> p x f", f=P)
    dA3 = dwA.rearrange("p (x f) -> p x f", f=P)
    dB3 = dwB.rearrange("p (x f) -> p x f", f=P)
    xtA3 = xtA.rearrange("p (x c) -> p x c", c=chunkA)
    xwA3 = xwA.rearrange("p (x c) -> p x c", c=chunkA)
    xtB3 = xtB.rearrange("p (x c) -> p x c", c=chunkB)
    xwB3 = xwB.rearrange("p (x c) -> p x c", c=chunkB)
    ytA3 = ytA.rearrange("p (x c) -> p x c", c=chunkA)
    ytB3 = ytB.rearrange("p (x c) -> p x c", c=chunkB)

    with tile.TileContext(nc) as tc:
        with (
            tc.tile_pool(name="big", bufs=1) as big,
            tc.tile_pool(name="sp", bufs=4) as sp,
            tc.tile_pool(name="yp", bufs=4) as yp,
            tc.tile_pool(name="pg", bufs=2, space="PSUM") as pgp,
            tc.tile_pool(name="pu", bufs=2, space="PSUM") as pup,
            tc.tile_pool(name="pd", bufs=3, space="PSUM") as pdp,
            tc.tile_pool(name="pw", bufs=1, space="PSUM") as pwp,
        ):
            # ── persistent tiles ────────────────────────────────────────
            wkt = big.tile([P, P], BF16, tag="wkt")
            xtsA = big.tile([P, nchA * DO, chunkA], BF16, tag="xtsA")
            xwsA = big.tile([P, nchA * DO, chunkA], BF16, tag="xwsA")
            xtsB = big.tile([P, nchB * DO, chunkB], BF16, tag="xtsB")
            xwsB = big.tile([P, nchB * DO, chunkB], BF16, tag="xwsB")
            gtA = big.tile([P, FO_H * DO, P], BF16, tag="gtA")
            utA = big.tile([P, FO_H * DO, P], BF16, tag="utA")
            gtB = big.tile([P, FO_H * DO, P], BF16, tag="gtB")
            utB = big.tile([P, FO_H * DO, P], BF16, tag="utB")
            dtA = big.tile([P, DO * FO_H, P], BF16, tag="dtA")
            dtB = big.tile([P, DO * FO_H, P], BF16, tag="dtB")
            hsA = big.tile([P, FO_H, CA], BF16, tag="hsA")
            hsB = big.tile([P, FO_H, CB], BF16, tag="hsB")

            # ── DMA schedule ────────────────────────────────────────────
            # queue G (gpsimd): warm tile, A tokens chunk0, A gate/up weight
            # slabs (interleaved, fo order), A down weights, B gate/up, B down.
            # queue S (sync): remaining A token chunks, B tokens.
            WS = 2 * DO  # weight slab: 2 f-tiles = [P, 16, P] (0.5 MB)
            nc.gpsimd.dma_start(wkt[:], wk)
            nc.gpsimd.dma_start(xtsA[:, 0:DO], xtA3[:, 0:DO])
            nc.gpsimd.dma_start(gtA[:, 0:WS], gA3[:, 0:WS])
            nc.gpsimd.dma_start(xwsA[:, 0:DO], xwA3[:, 0:DO])
            nc.gpsimd.dma_start(utA[:, 0:WS], uA3[:, 0:WS])
            for s in range(1, FO_H * DO // WS):
                ssl = slice(s * WS, (s + 1) * WS)
                nc.gpsimd.dma_start(gtA[:, ssl], gA3[:, ssl])
                nc.gpsimd.dma_start(utA[:, ssl], uA3[:, ssl])
            for cc in range(1, nchA):
                csl = slice(cc * DO, (cc + 1) * DO)
                nc.sync.dma_start(xtsA[:, csl], xtA3[:, csl])
                nc.sync.dma_start(xwsA[:, csl], xwA3[:, csl])
            for cc in range(nchB):
                csl = slice(cc * DO, (cc + 1) * DO)
                nc.sync.dma_start(xtsB[:, csl], xtB3[:, csl])
                nc.sync.dma_start(xwsB[:, csl], xwB3[:, csl])
            for s in range(2):
                ssl = slice(s * 32, (s + 1) * 32)
                nc.gpsimd.dma_start(dtA[:, ssl], dA3[:, ssl])
            for s in range(FO_H * DO // WS):
                ssl = slice(s * WS, (s + 1) * WS)
                nc.gpsimd.dma_start(gtB[:, ssl], gB3[:, ssl])
                nc.gpsimd.dma_start(utB[:, ssl], uB3[:, ssl])
            for s in range(2):
                ssl = slice(s * 32, (s + 1) * 32)
                nc.gpsimd.dma_start(dtB[:, ssl], dB3[:, ssl])

            # ── PE warmup: dummy matmuls while DMAs land ────────────────
            wps = pwp.tile([P, P], F32, tag="wps")
            for _ in range(N_WARM):
                nc.tensor.matmul(wps[:], wkt[:], wkt[:], start=True, stop=True)

            # ── task A, then task B (A first: its DMAs land first) ──────
            _task_phase1(nc, gtA, utA, xtsA, xwsA, hsA, (sp, pgp, pup), chunkA, nchA, "A")
            _task_phase2(nc, dtA, hsA, ytA3, (yp, pdp), chunkA, nchA, "A")
            _task_phase1(nc, gtB, utB, xtsB, xwsB, hsB, (sp, pgp, pup), chunkB, nchB, "B")
            _task_phase2(nc, dtB, hsB, ytB3, (yp, pdp), chunkB, nchB, "B")
    nc.finalize()
    return nc


def _get_program(key):
    if key not in _BUILD_CACHE:
        _BUILD_CACHE[key] = _build(*key)
    return _BUILD_CACHE[key]


def _sigmoid(z):
    return 1.0 / (1.0 + np.exp(-z))


def _route(xf32, macro_w, micro_w):
    """Host routers in float64. Returns group index per token and per-token
    weights for the 2 experts of the selected group (float32)."""
    xf = xf32.astype(np.float64)
    ms = _sigmoid(xf @ macro_w.astype(np.float64))  # [T, G]
    g_sel = np.argmax(ms, axis=1)
    T = xf.shape[0]
    mval = ms[np.arange(T), g_sel]
    mv = mval / (mval + EPS)

    w2 = np.zeros((T, 2), np.float64)
    for g in range(NUM_GROUPS):
        idx = np.nonzero(g_sel == g)[0]
        if idx.size == 0:
            continue
        s = _sigmoid(xf[idx] @ micro_w[g].astype(np.float64))  # [n, 2]
        denom = np.maximum(s[:, 0], s[:, 1]) + np.minimum(s[:, 0], s[:, 1]) + EPS
        w2[idx, 0] = mv[idx] * s[:, 0] / denom
        w2[idx, 1] = mv[idx] * s[:, 1] / denom
    return g_sel, w2.astype(np.float32)


def _pick_capacity(n: int):
    n = max(n, 16)
    nch = max(1, -(-n // 512))
    chunk = -(-n // nch)
    chunk = -(-chunk // 16) * 16
    return chunk * nch, nch


def _pack_w(w):
    """[1024, 1024] f32 -> [128, 8192] bf16, layout [p, outer, inner, 128]."""
    return np.ascontiguousarray(
        w.reshape(8, P, 8, P).transpose(1, 2, 0, 3).reshape(P, 8192)
    ).astype(ml_dtypes.bfloat16)


def _pack_x(xg, C, nch, chunk):
    """tokens [n, 1024] f32 -> [128, 8*C] bf16, layout [p, cc, do, c]."""
    n = xg.shape[0]
    z = np.zeros((D_MODEL, C), np.float32)
    if n:
        z[:, :n] = xg.T
    return np.ascontiguousarray(
        z.reshape(DO, P, nch, chunk).transpose(1, 2, 0, 3).reshape(P, DO * C)
    ).astype(ml_dtypes.bfloat16)


def _unpack_y(r, C, nch, chunk, n):
    """[128, 8*C] bf16 -> [n, 1024] f32 token-major partial output."""
    y = (
        np.asarray(r, dtype=np.float32)
        .reshape(P, nch, DO, chunk)
        .transpose(2, 0, 1, 3)
        .reshape(D_MODEL, C)
    )
    return y[:, :n].T


def kernel(x, macro_w, micro_w, gate_w, up_w, down_w):
    global LAST_RESULTS
    x = np.asarray(x)
    B, S, D = x.shape
    T = B * S
    xf = np.ascontiguousarray(x.reshape(T, D).astype(np.float32, copy=False))

    g_sel, w2 = _route(xf, np.asarray(macro_w), np.asarray(micro_w))
    idx_by_g = [np.nonzero(g_sel == g)[0] for g in range(NUM_GROUPS)]
    sizes = np.array([ix.size for ix in idx_by_g])

    # heavy groups -> task A, light groups -> task B (balance per-core load)
    order = np.argsort(-sizes, kind="stable")
    heavy = [int(order[0]), int(order[1])]
    light = [int(order[3]), int(order[2])]  # pair heaviest with lightest
    CA, nchA = _pick_capacity(int(sizes[heavy[0]]))
    CB, nchB = _pick_capacity(int(max(sizes[light[0]], sizes[light[1]])))
    chunkA, chunkB = CA // nchA, CB // nchB

    nc = _get_program((CA, nchA, CB, nchB))

    gate_w = np.asarray(gate_w, np.float32)
    up_w = np.asarray(up_w, np.float32)
    down_w = np.asarray(down_w, np.float32)

    wk0 = np.zeros((P, P), ml_dtypes.bfloat16)
    in_maps = []
    for c in range(N_CORES):
        j = (c // 2) % 2  # local expert within group
        h = c % 2  # F-half
        fsl = slice(h * F_HALF, (h + 1) * F_HALF)
        m = {"wk": wk0}
        for t, g_of, (C, nch, chunk) in (
            ("A", heavy[c // 4], (CA, nchA, chunkA)),
            ("B", light[c // 4], (CB, nchB, chunkB)),
        ):
            e = 2 * g_of + j
            ix = idx_by_g[g_of]
            xg = xf[ix]
            m["xt" + t] = _pack_x(xg, C, nch, chunk)
            m["xw" + t] = _pack_x(xg * w2[ix, j : j + 1], C, nch, chunk)
            m["gw" + t] = _pack_w(gate_w[e][:, fsl])
            m["uw" + t] = _pack_w(up_w[e][:, fsl])
            m["dw" + t] = _pack_w(down_w[e][fsl, :])
        in_maps.append(m)

    res = run_bass_kernel_spmd(nc, in_maps, core_ids=list(range(N_CORES)))
    LAST_RESULTS = res

    y = np.zeros((T, D), np.float32)
    for c in range(N_CORES):
        for t, g_of, (C, nch, chunk) in (
            ("A", heavy[c // 4], (CA, nchA, chunkA)),
            ("B", light[c // 4], (CB, nchB, chunkB)),
        ):
            ix = idx_by_g[g_of]
            if ix.size:
                y[ix] += _unpack_y(res.results[c]["yt" + t], C, nch, chunk, ix.size)
    return y.reshape(B, S, D)


# revision 9
# speedup vs baseline: 1.1763x; 1.0698x over previous
"""MoE FFN (grouped top-1 routing, SwiGLU experts) on 8 Trainium2 NeuronCores.

Strategy (expert-parallel with half-expert load balancing):
  - Host computes the routers (sigmoid macro top-1 group of 4; both experts of
    the selected group active with normalized sigmoid weights). Per-expert
    weight folded into the up-projection input (x*w) on the host.
  - Each expert's FFN dim F=2048 is split into two half-experts (F_half=1024).
    The 16 half-experts are placed so every core gets one half-expert from a
    HEAVY group (task A) and one from a LIGHT group (task B): heavy groups are
    the two with most routed tokens. This balances per-core work to
    ~(s0+s2)/2 tokens instead of max-group tokens.
  - Device kernel per core: two independent SwiGLU half-FFN tasks,
    Y_h^T = down_h^T @ (silu(gate_h^T X^T) * (up_h^T Xw^T)). bf16 storage,
    fp32 PSUM accumulation. Host sums the 4 half-partials per token.
  - DMA is explicitly staged (tokens + first weight slabs first, down-proj
    weights prefetched during phase 1) and the PE is pre-warmed with dummy
    matmuls so the HAM clock gate reaches 2.4 GHz before real work arrives.
"""

import math

import ml_dtypes
import numpy as np

import concourse.bass as bass  # noqa: F401  (bass types via bacc)
import concourse.mybir as mybir
import concourse.tile as tile
from concourse import bacc
from concourse.bass_utils import run_bass_kernel_spmd

P = 128
D_MODEL = 1024
FFN_DIM = 2048
F_HALF = FFN_DIM // 2
NUM_EXPERTS = 8
NUM_GROUPS = 4
EPS = 1e-9
DO = D_MODEL // P  # 8 k-tiles over D
FO_H = F_HALF // P  # 8 f-tiles per half-expert

F32 = mybir.dt.float32
BF16 = mybir.dt.bfloat16

N_CORES = 8
N_WARM = 28  # PE warmup matmuls (HAM un-throttle)

_BUILD_CACHE: dict[tuple, object] = {}
LAST_RESULTS = None  # stashed BassKernelResults for test harnesses


def _task_phase1(nc, gt3, ut3, xts3, xws3, hs, pools, chunk, nch, tag):
    """SwiGLU up to h = silu(gate^T x) * (up^T xw) for one half-expert task."""
    sp, pgp, pup = pools
    for cc in range(nch):
        cs = slice(cc * chunk, (cc + 1) * chunk)
        for fo in range(FO_H):
            psg = pgp.tile([P, 512], F32, tag="psg", name=f"psg{tag}_{cc}_{fo}")[:, :chunk]
            psu = pup.tile([P, 512], F32, tag="psu", name=f"psu{tag}_{cc}_{fo}")[:, :chunk]
            for do in range(DO):
                nc.tensor.matmul(
                    psg[:],
                    gt3[:, fo * DO + do, :],
                    xts3[:, cc * DO + do, :],
                    start=(do == 0),
                    stop=(do == DO - 1),
                )
            for do in range(DO):
                nc.tensor.matmul(
                    psu[:],
                    ut3[:, fo * DO + do, :],
                    xws3[:, cc * DO + do, :],
                    start=(do == 0),
                    stop=(do == DO - 1),
                )
            sg = sp.tile([P, chunk], F32, tag=f"sg{tag}", name=f"sg{tag}_{cc}_{fo}")
            nc.scalar.activation(sg[:], psg[:], mybir.ActivationFunctionType.Silu)
            nc.vector.tensor_mul(out=hs[:, fo, cs], in0=sg[:], in1=psu[:])


def _task_phase2(nc, dt3, hs, yt3, pools, chunk, nch, tag):
    """y^T = down_h^T @ h for one half-expert task; stream bf16 output out."""
    yp, pdp = pools
    for cc in range(nch):
        cs = slice(cc * chunk, (cc + 1) * chunk)
        for do in range(DO):
            psy = pdp.tile([P, 512], F32, tag="psy", name=f"psy{tag}_{cc}_{do}")[:, :chunk]
            for fo in range(FO_H):
                nc.tensor.matmul(
                    psy[:],
                    dt3[:, do * FO_H + fo, :],
                    hs[:, fo, cs],
                    start=(fo == 0),
                    stop=(fo == FO_H - 1),
                )
            yo = yp.tile([P, chunk], BF16, tag=f"yo{tag}", name=f"yo{tag}_{cc}_{do}")
            nc.any.tensor_copy(out=yo[:], in_=psy[:])
            nc.scalar.dma_start(yt3[:, cc * DO + do, :], yo[:])


def _build(CA: int, nchA: int, CB: int, nchB: int):
    """Bass/Tile program: two half-expert SwiGLU tasks per core."""
    chunkA, chunkB = CA // nchA, CB // nchB
    assert chunkA * nchA == CA and chunkA <= 512
    assert chunkB * nchB == CB and chunkB <= 512

    nc = bacc.Bacc(
        "TRN2",
        target_bir_lowering=False,
        debug=False,
        enable_asserts=False,
        num_devices=N_CORES,
    )
    xtA = nc.dram_tensor("xtA", [P, DO * CA], BF16, kind="ExternalInput").ap()
    xwA = nc.dram_tensor("xwA", [P, DO * CA], BF16, kind="ExternalInput").ap()
    xtB = nc.dram_tensor("xtB", [P, DO * CB], BF16, kind="ExternalInput").ap()
    xwB = nc.dram_tensor("xwB", [P, DO * CB], BF16, kind="ExternalInput").ap()
    gwA = nc.dram_tensor("gwA", [P, FO_H * DO * P], BF16, kind="ExternalInput").ap()
    uwA = nc.dram_tensor("uwA", [P, FO_H * DO * P], BF16, kind="ExternalInput").ap()
    gwB = nc.dram_tensor("gwB", [P, FO_H * DO * P], BF16, kind="ExternalInput").ap()
    uwB = nc.dram_tensor("uwB", [P, FO_H * DO * P], BF16, kind="ExternalInput").ap()
    dwA = nc.dram_tensor("dwA", [P, DO * FO_H * P], BF16, kind="ExternalInput").ap()
    dwB = nc.dram_tensor("dwB", [P, DO * FO_H * P], BF16, kind="ExternalInput").ap()
    ytA = nc.dram_tensor("ytA", [P, DO * CA], BF16, kind="ExternalOutput").ap()
    ytB = nc.dram_tensor("ytB", [P, DO * CB], BF16, kind="ExternalOutput").ap()

    # packed views: weights [p, idx, 128], tokens/outputs [p, idx, chunk]
    gA3 = gwA.rearrange("p (x f) -> p x f", f=P)
    uA3 = uwA.rearrange("p (x f) -> p x f", f=P)
    gB3 = gwB.rearrange("p (x f) -> p x f", f=P)
    uB3 = uwB.rearrange("p (x f) -> p x f", f=P)
    dA3 = dwA.rearrange("p (x f) -> p x f", f=P)
    dB3 = dwB.rearrange("p (x f) -> p x f", f=P)
    xtA3 = xtA.rearrange("p (x c) -> p x c", c=chunkA)
    xwA3 = xwA.rearrange("p (x c) -> p x c", c=chunkA)
    xtB3 = xtB.rearrange("p (x c) -> p x c", c=chunkB)
    xwB3 = xwB.rearrange("p (x c) -> p x c", c=chunkB)
    ytA3 = ytA.rearrange("p (x c) -> p x c", c=chunkA)
    ytB3 = ytB.rearrange("p (x c) -> p x c", c=chunkB)

    with tile.TileContext(nc) as tc:
        with (
            tc.tile_pool(name="big", bufs=1) as big,
            tc.tile_pool(name="sp", bufs=4) as sp,
            tc.tile_pool(name="yp", bufs=4) as yp,
            tc.tile_pool(name="pg", bufs=2, space="PSUM") as pgp,
            tc.tile_pool(name="pu", bufs=2, space="PSUM") as pup,
            tc.tile_pool(name="pd", bufs=3, space="PSUM") as pdp,
            tc.tile_pool(name="pw", bufs=1, space="PSUM") as pwp,
        ):
            # ── persistent tiles ────────────────────────────────────────
            wkt = big.tile([P, P], BF16, tag="wkt")
            xtsA = big.tile([P, nchA * DO, chunkA], BF16, tag="xtsA")
            xwsA = big.tile([P, nchA * DO, chunkA], BF16, tag="xwsA")
            xtsB = big.tile([P, nchB * DO, chunkB], BF16, tag="xtsB")
            xwsB = big.tile([P, nchB * DO, chunkB], BF16, tag="xwsB")
            gtA = big.tile([P, FO_H * DO, P], BF16, tag="gtA")
            utA = big.tile([P, FO_H * DO, P], BF16, tag="utA")
            gtB = big.tile([P, FO_H * DO, P], BF16, tag="gtB")
            utB = big.tile([P, FO_H * DO, P], BF16, tag="utB")
            dtA = big.tile([P, DO * FO_H, P], BF16, tag="dtA")
            dtB = big.tile([P, DO * FO_H, P], BF16, tag="dtB")
            hsA = big.tile([P, FO_H, CA], BF16, tag="hsA")
            hsB = big.tile([P, FO_H, CB], BF16, tag="hsB")

            # ── PE warmup: memset a tile (no DMA dep), run dummy matmuls
            # so the HAM clock gate reaches 2.4 GHz before real work ──────
            nc.vector.memset(wkt[:], 0.25)
            wps = pwp.tile([P, P], F32, tag="wps")
            for _ in range(N_WARM):
                nc.tensor.matmul(wps[:], wkt[:], wkt[:], start=True, stop=True)

            # ── DMA schedule: ONE input queue (gpsimd), strict priority
            # order so early compute is never starved by bulk transfers ──
            def _slabs(dst, src, plan):
                for lo, hi in plan:
                    nc.gpsimd.dma_start(dst[:, lo * DO : hi * DO], src[:, lo * DO : hi * DO])

            FINE = [(0, 1), (1, 2), (2, 4), (4, 6), (6, 8)]
            nc.gpsimd.dma_start(xtsA[:, 0:DO], xtA3[:, 0:DO])
            _slabs(gtA, gA3, FINE[:1])
            _slabs(utA, uA3, FINE[:1])
            nc.gpsimd.dma_start(xwsA[:, 0:DO], xwA3[:, 0:DO])
            for lo, hi in FINE[1:3]:
                _slabs(gtA, gA3, [(lo, hi)])
                _slabs(utA, uA3, [(lo, hi)])
            for cc in range(1, nchA):
                csl = slice(cc * DO, (cc + 1) * DO)
                nc.gpsimd.dma_start(xtsA[:, csl], xtA3[:, csl])
                nc.gpsimd.dma_start(xwsA[:, csl], xwA3[:, csl])
            for lo, hi in FINE[3:]:
                _slabs(gtA, gA3, [(lo, hi)])
                _slabs(utA, uA3, [(lo, hi)])
            for s in range(2):
                ssl = slice(s * 32, (s + 1) * 32)
                nc.gpsimd.dma_start(dtA[:, ssl], dA3[:, ssl])
            for cc in range(nchB):
                csl = slice(cc * DO, (cc + 1) * DO)
                nc.gpsimd.dma_start(xtsB[:, csl], xtB3[:, csl])
                nc.gpsimd.dma_start(xwsB[:, csl], xwB3[:, csl])
            for lo, hi in [(0, 4), (4, 8)]:
                _slabs(gtB, gB3, [(lo, hi)])
                _slabs(utB, uB3, [(lo, hi)])
            for s in range(2):
                ssl = slice(s * 32, (s + 1) * 32)
                nc.gpsimd.dma_start(dtB[:, ssl], dB3[:, ssl])

            # ── task A, then task B (A first: its DMAs land first) ──────
            _task_phase1(nc, gtA, utA, xtsA, xwsA, hsA, (sp, pgp, pup), chunkA, nchA, "A")
            _task_phase2(nc, dtA, hsA, ytA3, (yp, pdp), chunkA, nchA, "A")
            _task_phase1(nc, gtB, utB, xtsB, xwsB, hsB, (sp, pgp, pup), chunkB, nchB, "B")
            _task_phase2(nc, dtB, hsB, ytB3, (yp, pdp), chunkB, nchB, "B")
    nc.finalize()
    return nc


def _get_program(key):
    if key not in _BUILD_CACHE:
        _BUILD_CACHE[key] = _build(*key)
    return _BUILD_CACHE[key]


def _sigmoid(z):
    return 1.0 / (1.0 + np.exp(-z))


def _route(xf32, macro_w, micro_w):
    """Host routers in float64. Returns group index per token and per-token
    weights for the 2 experts of the selected group (float32)."""
    xf = xf32.astype(np.float64)
    ms = _sigmoid(xf @ macro_w.astype(np.float64))  # [T, G]
    g_sel = np.argmax(ms, axis=1)
    T = xf.shape[0]
    mval = ms[np.arange(T), g_sel]
    mv = mval / (mval + EPS)

    w2 = np.zeros((T, 2), np.float64)
    for g in range(NUM_GROUPS):
        idx = np.nonzero(g_sel == g)[0]
        if idx.size == 0:
            continue
        s = _sigmoid(xf[idx] @ micro_w[g].astype(np.float64))  # [n, 2]
        denom = np.maximum(s[:, 0], s[:, 1]) + np.minimum(s[:, 0], s[:, 1]) + EPS
        w2[idx, 0] = mv[idx] * s[:, 0] / denom
        w2[idx, 1] = mv[idx] * s[:, 1] / denom
    return g_sel, w2.astype(np.float32)


def _pick_capacity(n: int):
    n = max(n, 16)
    nch = max(1, -(-n // 512))
    chunk = -(-n // nch)
    chunk = -(-chunk // 16) * 16
    return chunk * nch, nch


def _pack_w(w):
    """[1024, 1024] f32 -> [128, 8192] bf16, layout [p, outer, inner, 128]."""
    return np.ascontiguousarray(
        w.reshape(8, P, 8, P).transpose(1, 2, 0, 3).reshape(P, 8192)
    ).astype(ml_dtypes.bfloat16)


def _pack_x(xg, C, nch, chunk):
    """tokens [n, 1024] f32 -> [128, 8*C] bf16, layout [p, cc, do, c]."""
    n = xg.shape[0]
    z = np.zeros((D_MODEL, C), np.float32)
    if n:
        z[:, :n] = xg.T
    return np.ascontiguousarray(
        z.reshape(DO, P, nch, chunk).transpose(1, 2, 0, 3).reshape(P, DO * C)
    ).astype(ml_dtypes.bfloat16)


def _unpack_y(r, C, nch, chunk, n):
    """[128, 8*C] bf16 -> [n, 1024] f32 token-major partial output."""
    y = (
        np.asarray(r, dtype=np.float32)
        .reshape(P, nch, DO, chunk)
        .transpose(2, 0, 1, 3)
        .reshape(D_MODEL, C)
    )
    return y[:, :n].T


def kernel(x, macro_w, micro_w, gate_w, up_w, down_w):
    global LAST_RESULTS
    x = np.asarray(x)
    B, S, D = x.shape
    T = B * S
    xf = np.ascontiguousarray(x.reshape(T, D).astype(np.float32, copy=False))

    g_sel, w2 = _route(xf, np.asarray(macro_w), np.asarray(micro_w))
    idx_by_g = [np.nonzero(g_sel == g)[0] for g in range(NUM_GROUPS)]
    sizes = np.array([ix.size for ix in idx_by_g])

    # heavy groups -> task A, light groups -> task B (balance per-core load)
    order = np.argsort(-sizes, kind="stable")
    heavy = [int(order[0]), int(order[1])]
    light = [int(order[3]), int(order[2])]  # pair heaviest with lightest
    CA, nchA = _pick_capacity(int(sizes[heavy[0]]))
    CB, nchB = _pick_capacity(int(max(sizes[light[0]], sizes[light[1]])))
    chunkA, chunkB = CA // nchA, CB // nchB

    nc = _get_program((CA, nchA, CB, nchB))

    gate_w = np.asarray(gate_w, np.float32)
    up_w = np.asarray(up_w, np.float32)
    down_w = np.asarray(down_w, np.float32)

    in_maps = []
    for c in range(N_CORES):
        j = (c // 2) % 2  # local expert within group
        h = c % 2  # F-half
        fsl = slice(h * F_HALF, (h + 1) * F_HALF)
        m = {}
        for t, g_of, (C, nch, chunk) in (
            ("A", heavy[c // 4], (CA, nchA, chunkA)),
            ("B", light[c // 4], (CB, nchB, chunkB)),
        ):
            e = 2 * g_of + j
            ix = idx_by_g[g_of]
            xg = xf[ix]
            m["xt" + t] = _pack_x(xg, C, nch, chunk)
            m["xw" + t] = _pack_x(xg * w2[ix, j : j + 1], C, nch, chunk)
            m["gw" + t] = _pack_w(gate_w[e][:, fsl])
            m["uw" + t] = _pack_w(up_w[e][:, fsl])
            m["dw" + t] = _pack_w(down_w[e][fsl, :])
        in_maps.append(m)

    res = run_bass_kernel_spmd(nc, in_maps, core_ids=list(range(N_CORES)))
    LAST_RESULTS = res

    y = np.zeros((T, D), np.float32)
    for c in range(N_CORES):
        for t, g_of, (C, nch, chunk) in (
            ("A", heavy[c // 4], (CA, nchA, chunkA)),
            ("B", light[c // 4], (CB, nchB, chunkB)),
        ):
            ix = idx_by_g[g_of]
            if ix.size:
                y[ix] += _unpack_y(res.results[c]["yt" + t], C, nch, chunk, ix.size)
    return y.reshape(B, S, D)
